# revision 8
# baseline (speedup 1.0000x reference)
"""Trainium2 Bass kernel for the gated-GNN message-passing module.

Math per step (per batch b):
    pre_e = h @ W_e                              [N, D]   e in 0..2E
    m     = sum_e A_e @ pre_e + C_b              [N, D]   (C_b folds the b_msg bias)
    rz    = sigmoid(m @ wih[:, :2D] + h @ whh[:, :2D] + b_rz)
    n     = tanh(m @ wih[:, 2D:] + b_in + r * (h @ whh[:, 2D:] + b_hn))
    h     = n + z * (h - n)

Everything on-chip is kept TRANSPOSED (h^T [D, N], m^T [D, N], gates [3D, N]) so
that gate biases are per-partition vectors and no transposes are needed anywhere:
  pre     : out[src, ef]   = (h^T)^T @ Wcat           (lhsT = h^T tile)
  m^T     : out[d, dst]   += pre^T @ A_full_e^T       (lhsT = pre tile, rhs = R_e)
  gates^T : out[g, node]  += w^T @ {m^T, h^T}         (lhsT = w tile)
R_e (the aggregation rhs) is A_full_e^T: adj_e^T for forward edges, adj_{e-4} for
backward edges; both prepared host-side, bf16.

Sharding: data-parallel over batch. B=32 -> 4 batches per core x 8 cores.
Two batches are interleaved in program order so one batch's gate chain
(DVE/ACT) hides under the other batch's matmuls (PE).
"""

import os

# Adjacency tiles are each read by 64 matmuls before their slot is reused;
# with per-subtile dependency tracking the reuse DMA accumulates more sync
# waits than a DMA descriptor supports ("Too many sync wait commands" in
# walrus codegen). Whole-tile dependency granularity avoids that.
os.environ.setdefault("BY_DEFAULT_DISABLE_SUBTILE_DEPS", "1")

import numpy as np
import ml_dtypes

import concourse.bass as bass
import concourse.bacc as bacc
import concourse.mybir as mybir
import concourse.tile as tile
from concourse.bass_utils import run_bass_kernel_spmd

BF16 = mybir.dt.bfloat16
F32 = mybir.dt.float32
AF = mybir.ActivationFunctionType
ALU = mybir.AluOpType

B, E, N, D, V = 32, 4, 512, 256, 1024
E2 = 2 * E          # 8 edge types incl. backward
NB = B // 8         # batches per core = 4
NT = N // 128       # node-dim tiles = 4
KD = D // 128       # feature-dim tiles = 2
P = 128


def _step(nc, b, si, adj_sb, hT_f, hT_b, CbT_sb, W, pools):
    """Emit one propagation step for batch b. Returns (new hT_f, new hT_b)."""
    wc_sb, wih_sb, whh_sb, bias_sb = W
    pre_pool, gpool, hpool, ps_pre, ps_mt, ps_gru = pools

    # ---- phase 1: pre[src, e*D+f] (no bias; b_msg folded into CbT) ----
    pre_sb = []
    for s in range(NT):
        t = pre_pool.tile([P, E2 * D], BF16, name=f"pre_b{b}s{si}_{s}", tag="pre")
        pre_sb.append(t)
        for nch in range(4):  # 4 x 512 columns of (e, f)
            pp = ps_pre.tile([P, 512], F32, name=f"pp_b{b}s{si}_{s}_{nch}", tag="pp")
            for k in range(KD):
                nc.tensor.matmul(
                    pp[:],
                    lhsT=hT_b[k][:, s * P:(s + 1) * P],
                    rhs=wc_sb[k][:, nch * 512:(nch + 1) * 512],
                    start=(k == 0),
                    stop=(k == KD - 1),
                )
            # cast fp32 PSUM -> bf16 SBUF; alternate engines to balance load
            dst = t[:, nch * 512:(nch + 1) * 512]
            if nch % 2 == 0:
                nc.scalar.copy(dst, pp[:])
            else:
                # tensor_copy lowers to a fast-path encoding (S4D4_TR) with a
                # single sync-wait slot; tensor_scalar_add(x, 0) is wait-safe.
                nc.vector.tensor_scalar_add(dst, pp[:], 0.0)

    # ---- phase 2: m^T[d, dst] accumulated over (s, e) ----
    mt_ps = []
    for d_ in range(KD):
        t = ps_mt.tile([P, N], F32, name=f"mt_b{b}s{si}_{d_}", tag="mt")
        mt_ps.append(t)
    n_acc = NT * E2
    for s in range(NT):
        for e in range(E2):
            first = (s == 0 and e == 0)
            last = (s == NT - 1 and e == E2 - 1)
            for d_ in range(KD):
                nc.tensor.matmul(
                    mt_ps[d_][:],
                    lhsT=pre_sb[s][:, e * D + d_ * P: e * D + (d_ + 1) * P],
                    rhs=adj_sb[e][:, s * 512:(s + 1) * 512],
                    start=first,
                    stop=last,
                )
    # m^T + C_b^T -> bf16 for the GRU matmuls (DVE reads PSUM)
    mt_b = []
    for d_ in range(KD):
        t = gpool.tile([P, N], BF16, name=f"mtb_b{b}s{si}_{d_}", tag="mtb", bufs=4)
        nc.vector.tensor_add(t[:], mt_ps[d_][:], CbT_sb[d_][:])
        mt_b.append(t)

    # ---- phase 3+4: GRU gates, all in transposed [gate_dim, node] layout ----
    # r/z: gates 0..511; accumulate both w_ih^T@m^T and w_hh^T@h^T in PSUM
    rz_sb = []
    for t_ in range(4):
        pg = ps_gru.tile([P, N], F32, name=f"rz_b{b}s{si}_{t_}", tag="pg")
        for k in range(KD):
            nc.tensor.matmul(pg[:], lhsT=wih_sb[k][:, t_ * P:(t_ + 1) * P],
                             rhs=mt_b[k][:], start=(k == 0), stop=False)
            nc.tensor.matmul(pg[:], lhsT=whh_sb[k][:, t_ * P:(t_ + 1) * P],
                             rhs=hT_b[k][:], start=False, stop=(k == KD - 1))
        g = gpool.tile([P, N], F32, name=f"g_b{b}s{si}_{t_}", tag="rz")
        nc.scalar.activation(g[:], pg[:], AF.Sigmoid, bias=bias_sb[:, t_:t_ + 1])
        rz_sb.append(g)
    r_sb, z_sb = rz_sb[:2], rz_sb[2:]

    new_hf, new_hb = [], []
    for t_ in range(KD):
        pin = ps_gru.tile([P, N], F32, name=f"pin_b{b}s{si}_{t_}", tag="pg")
        for k in range(KD):
            nc.tensor.matmul(pin[:], lhsT=wih_sb[k][:, 2 * D + t_ * P: 2 * D + (t_ + 1) * P],
                             rhs=mt_b[k][:], start=(k == 0), stop=(k == KD - 1))
        phn = ps_gru.tile([P, N], F32, name=f"phn_b{b}s{si}_{t_}", tag="pg")
        for k in range(KD):
            nc.tensor.matmul(phn[:], lhsT=whh_sb[k][:, 2 * D + t_ * P: 2 * D + (t_ + 1) * P],
                             rhs=hT_b[k][:], start=(k == 0), stop=(k == KD - 1))
        # rhn = (h_n + b_hn) * r      (fused)
        rhn = gpool.tile([P, N], F32, name=f"rhn_b{b}s{si}_{t_}", tag="tmp")
        nc.vector.scalar_tensor_tensor(rhn[:], phn[:], bias_sb[:, 6 + t_:7 + t_],
                                       r_sb[t_][:], op0=ALU.add, op1=ALU.mult)
        # tsum = (i_n + b_in) + rhn   (fused)
        tsum = gpool.tile([P, N], F32, name=f"ts_b{b}s{si}_{t_}", tag="tmp")
        nc.vector.scalar_tensor_tensor(tsum[:], pin[:], bias_sb[:, 4 + t_:5 + t_],
                                       rhn[:], op0=ALU.add, op1=ALU.add)
        nsb = gpool.tile([P, N], F32, name=f"n_b{b}s{si}_{t_}", tag="tmp")
        nc.scalar.activation(nsb[:], tsum[:], AF.Tanh)
        # h_new = n + z * (h - n)
        diff = gpool.tile([P, N], F32, name=f"df_b{b}s{si}_{t_}", tag="tmp")
        nc.vector.tensor_sub(diff[:], hT_f[t_][:], nsb[:])
        zd = gpool.tile([P, N], F32, name=f"zd_b{b}s{si}_{t_}", tag="tmp")
        nc.vector.tensor_mul(zd[:], z_sb[t_][:], diff[:])
        hf = hpool.tile([P, N], F32, name=f"hf_b{b}s{si}_{t_}", tag="hTf")
        nc.vector.tensor_add(hf[:], zd[:], nsb[:])
        hb = hpool.tile([P, N], BF16, name=f"hb_b{b}s{si}_{t_}", tag="hTb")
        nc.scalar.copy(hb[:], hf[:])
        new_hf.append(hf)
        new_hb.append(hb)
    return new_hf, new_hb


def build_nc(n_steps: int):
    # Bacc (not raw Bass): its compile() pass legalizes multi-semaphore waits
    # into the single wait slot the TPB instruction format provides.
    nc = bacc.Bacc("TRN2", target_bir_lowering=False, debug=False)
    R_d = nc.dram_tensor("Radj", [NB, E2, P, NT * 512], BF16, kind="ExternalInput").ap()
    hT0_d = nc.dram_tensor("hT0", [NB, KD, P, N], F32, kind="ExternalInput").ap()
    CbT_d = nc.dram_tensor("CbT", [NB, KD, P, N], F32, kind="ExternalInput").ap()
    Wc_d = nc.dram_tensor("Wcat", [KD, P, E2 * D], BF16, kind="ExternalInput").ap()
    Wih_d = nc.dram_tensor("Wih", [KD, P, 3 * D], BF16, kind="ExternalInput").ap()
    Whh_d = nc.dram_tensor("Whh", [KD, P, 3 * D], BF16, kind="ExternalInput").ap()
    Bias_d = nc.dram_tensor("BiasT", [P, 8], F32, kind="ExternalInput").ap()
    Out_d = nc.dram_tensor("hT_out", [NB, KD, P, N], F32, kind="ExternalOutput").ap()

    with tile.TileContext(nc) as tc:
        with (
            tc.tile_pool(name="wpool", bufs=1) as wpool,
            tc.tile_pool(name="adjp", bufs=22) as adj_pool,
            tc.tile_pool(name="prep", bufs=6) as pre_pool,
            tc.tile_pool(name="gp", bufs=6) as gpool,
            tc.tile_pool(name="hp", bufs=7) as hpool,
            tc.tile_pool(name="cbp", bufs=5) as cbpool,
            tc.tile_pool(name="pspre", bufs=2, space="PSUM") as ps_pre,
            tc.tile_pool(name="psmt", bufs=2, space="PSUM") as ps_mt,
            tc.tile_pool(name="psgru", bufs=3, space="PSUM") as ps_gru,
        ):
            # ---- weights, loaded once ----
            wc_sb, wih_sb, whh_sb = [], [], []
            for k in range(KD):
                wck = wpool.tile([P, E2 * D], BF16, name=f"wc{k}")
                nc.sync.dma_start(out=wck[:], in_=Wc_d[k])
                wc_sb.append(wck)
                wihk = wpool.tile([P, 3 * D], BF16, name=f"wih{k}")
                nc.sync.dma_start(out=wihk[:], in_=Wih_d[k])
                wih_sb.append(wihk)
                whhk = wpool.tile([P, 3 * D], BF16, name=f"whh{k}")
                nc.sync.dma_start(out=whhk[:], in_=Whh_d[k])
                whh_sb.append(whhk)
            bias_sb = wpool.tile([P, 8], F32, name="bias")
            nc.sync.dma_start(out=bias_sb[:], in_=Bias_d)
            W = (wc_sb, wih_sb, whh_sb, bias_sb)
            pools = (pre_pool, gpool, hpool, ps_pre, ps_mt, ps_gru)

            def load_adj(b):
                tiles = []
                for e in range(E2):
                    t = adj_pool.tile([P, NT * 512], BF16, name=f"adj{b}_{e}", tag="adj")
                    nc.sync.dma_start(out=t[:], in_=R_d[b, e])
                    tiles.append(t)
                return tiles

            def load_h(b):
                hf, hb, cb = [], [], []
                for k in range(KD):
                    f = hpool.tile([P, N], F32, name=f"h0f{b}_{k}", tag="hTf")
                    nc.sync.dma_start(out=f[:], in_=hT0_d[b, k])
                    bt = hpool.tile([P, N], BF16, name=f"h0b{b}_{k}", tag="hTb")
                    nc.scalar.copy(bt[:], f[:])
                    c = cbpool.tile([P, N], F32, name=f"cb{b}_{k}", tag="cb")
                    nc.sync.dma_start(out=c[:], in_=CbT_d[b, k])
                    hf.append(f)
                    hb.append(bt)
                    cb.append(c)
                return hf, hb, cb

            def store_h(b, hf):
                for k in range(KD):
                    nc.sync.dma_start(out=Out_d[b, k], in_=hf[k][:])

            npairs = NB // 2
            adj = {}
            state = {}
            for b in (0, 1):
                adj[b] = load_adj(b)
            if NB > 2:
                adj[2] = load_adj(2)  # prefetch into the third slot set
                adj[3] = load_adj(3)  # queued; waits on batch-0 slots freeing
            for pair in range(npairs):
                bA, bB = 2 * pair, 2 * pair + 1
                for b in (bA, bB):
                    state[b] = load_h(b)
                for si in range(n_steps):
                    for b in (bA, bB):
                        hf, hb, cb = state[b]
                        hf, hb = _step(nc, b, si, adj[b], hf, hb, cb, W, pools)
                        state[b] = (hf, hb, cb)
                for b in (bA, bB):
                    store_h(b, state[b][0])
    nc.compile()
    return nc


_NC_CACHE = {}


def _get_nc(n_steps: int):
    if n_steps not in _NC_CACHE:
        _NC_CACHE[n_steps] = build_nc(n_steps)
    return _NC_CACHE[n_steps]


def _prep_core_inputs(core, adj_tensor, hT0_all, CbT_all, shared):
    lo = core * NB
    sl = slice(lo, lo + NB)
    adj = adj_tensor[sl]                                # [NB, E, N, N] f32
    # R_e = A_full_e^T: forward edges -> adj^T, backward -> adj
    Rf = np.concatenate([adj.transpose(0, 1, 3, 2), adj], axis=1)  # [NB, 8, N, N]
    Radj = np.ascontiguousarray(
        Rf.reshape(NB, E2, NT, P, N).transpose(0, 1, 3, 2, 4)
    ).reshape(NB, E2, P, NT * N).astype(ml_dtypes.bfloat16)
    return {
        "Radj": Radj,
        "hT0": np.ascontiguousarray(hT0_all[sl]),
        "CbT": np.ascontiguousarray(CbT_all[sl]),
        **shared,
    }


def kernel(adj_tensor, node_labels, n_prop_steps, emb_table, W_msg, b_msg,
           w_ih, w_hh, b_ih, b_hh):
    adj_tensor = np.asarray(adj_tensor, dtype=np.float32)
    node_labels = np.asarray(node_labels)
    emb_table = np.asarray(emb_table, dtype=np.float32)
    W_msg = np.asarray(W_msg, dtype=np.float32)
    b_msg = np.asarray(b_msg, dtype=np.float32)
    w_ih = np.asarray(w_ih, dtype=np.float32)
    w_hh = np.asarray(w_hh, dtype=np.float32)
    b_ih = np.asarray(b_ih, dtype=np.float32)
    b_hh = np.asarray(b_hh, dtype=np.float32)
    n_steps = int(np.asarray(n_prop_steps))

    # ---- host-side marshaling ----
    h0 = emb_table[node_labels]                        # [B, N, D]
    hT0_all = h0.transpose(0, 2, 1).reshape(B, KD, P, N).astype(np.float32)
    # C_b = sum_e rowsum(A_full_e) outer b_msg_e  (the aggregated b_msg bias)
    rs_fwd = adj_tensor.sum(axis=3)                    # [B, E, N] rowsums of adj_e
    rs_bwd = adj_tensor.sum(axis=2)                    # [B, E, N] rowsums of adj_e^T
    rs = np.concatenate([rs_fwd, rs_bwd], axis=1)      # [B, 8, N]
    Cb = np.einsum("ben,ed->bnd", rs, b_msg)           # [B, N, D]
    CbT_all = Cb.transpose(0, 2, 1).reshape(B, KD, P, N).astype(np.float32)

    Wcat = np.ascontiguousarray(
        W_msg.reshape(E2, KD, P, D).transpose(1, 2, 0, 3)
    ).reshape(KD, P, E2 * D).astype(ml_dtypes.bfloat16)
    Wih = np.ascontiguousarray(w_ih.reshape(KD, P, 3 * D)).astype(ml_dtypes.bfloat16)
    Whh = np.ascontiguousarray(w_hh.reshape(KD, P, 3 * D)).astype(ml_dtypes.bfloat16)
    brz = (b_ih[:2 * D] + b_hh[:2 * D]).reshape(4, P)
    bin_ = b_ih[2 * D:].reshape(KD, P)
    bhn = b_hh[2 * D:].reshape(KD, P)
    BiasT = np.concatenate([brz, bin_, bhn], axis=0).T.astype(np.float32)  # [P, 8]
    BiasT = np.ascontiguousarray(BiasT)

    shared = {"Wcat": Wcat, "Wih": Wih, "Whh": Whh, "BiasT": BiasT}
    in_maps = [
        _prep_core_inputs(c, adj_tensor, hT0_all, CbT_all, shared)
        for c in range(8)
    ]

    nc = _get_nc(n_steps)
    res = run_bass_kernel_spmd(nc, in_maps, core_ids=list(range(8)))

    out = np.empty((B, N, D), dtype=np.float32)
    for c in range(8):
        hT = res.results[c]["hT_out"].reshape(NB, D, N)  # [NB, D, N]
        out[c * NB:(c + 1) * NB] = hT.transpose(0, 2, 1)
    return out
